# revision 24
# baseline (speedup 1.0000x reference)
"""Single-head attention kernel for Trainium2, SPMD over 8 NeuronCores.

Problem: x [4,4096,1024] f32 -> q/k/v = x@W+b (head 128) -> softmax(q k^T/sqrt(128)) @ v.
Sharding: core i handles batch i//2, query half i%2. Each core receives its
batch's x pre-transposed on the host to x^T [e, s] layout (and rotated so its
2048 queries are cols 0:2048; key order is irrelevant to softmax sums), so all
cores run one identical program and no on-chip transposes of x are needed.

Perf notes (from NTFF traces on this hardware):
- fp16 matmul is 1 cyc/row (fp32 is 4); everything is O(10) so the compute
  path runs fp16 with fp32 PSUM accumulation (measured ~5e-4 end-to-end).
- Host supplies x^T chunked as [sc*128+p, ec*512+c] and weights prepacked to
  [p, ec*128+c] f16 so every input is a single wide-row DMA.
- The PE p-state ramp (0.65/1.2 GHz until ~3us of continuous work) is burned
  by warmup matmuls on a memset tile while the first x DMA is in flight.
- exp on ScalarE costs ~(N+352)/1.2ns per instruction; total exp is ~75us/core
  and is THE critical rail: all 32 qb0 tiles plus the first 10 qb1 tiles are
  computed+exp'd inside phase 1 so ScalarE saturates from ~15us on.
- qb1's P@V is split into four 8-kt quarters with partial sums staged to SBUF
  (VectorE combines them); only the last quarter is gated by the final exp,
  cutting the post-exp tail from ~14us to ~5us. A pair-wise HBM AllReduce to
  split the duplicated K/V projection work was measured at ~45-50us for 1MB
  (plus ~7us extra teardown) and abandoned.
- P@V appends a ones-column to V so the softmax denominator lands in PSUM
  column 128 of each accumulator for free.
"""

import sys

if "/opt/trn_rl_repo" not in sys.path:
    sys.path.insert(0, "/opt/trn_rl_repo")

import numpy as np

P = 128          # partitions
S = 4096         # sequence length
E = 1024         # n_embd
D = 128          # head size
SQ = 2048        # queries per core
SC = 512         # s-processing chunk (phase 1)
NSC = S // SC    # 8
NEC = E // P     # 8
NKT = S // P     # 32 key tiles
QBLK = 1024      # query block (ACT instruction width)
NQB = SQ // QBLK # 2
N1PH1 = 10       # qb1 score tiles computed in phase 1

SCALE = 1.0 / float(np.sqrt(D))

_CACHE = {}


def _build_nc():
    import concourse.mybir as mybir
    import concourse.tile as tile
    from concourse import bacc

    f32 = mybir.dt.float32
    f16 = mybir.dt.float16
    AF = mybir.ActivationFunctionType

    nc = bacc.Bacc(None, target_bir_lowering=False)
    # xt[sc*128+p, ec*512+c] = x^T[ec*128+p, sc*512+c] (host pre-chunked)
    xt = nc.dram_tensor("xt", [S // SC * P, NEC * SC], f16, kind="ExternalInput")
    # weights host-prepacked to [p, ec*128+c] = W[ec*128+p, c], f16
    wq = nc.dram_tensor("wqp", [P, E], f16, kind="ExternalInput")
    wk = nc.dram_tensor("wkp", [P, E], f16, kind="ExternalInput")
    wv = nc.dram_tensor("wvp", [P, E], f16, kind="ExternalInput")
    bias = nc.dram_tensor("bias", [P, 3], f32, kind="ExternalInput")  # [bk|bv|bq]
    ident = nc.dram_tensor("ident", [P, P], f16, kind="ExternalInput")
    out = nc.dram_tensor("out", [SQ, D], f32, kind="ExternalOutput")

    with tile.TileContext(nc) as tc:
        with tc.tile_pool(name="big", bufs=1) as bigp, \
             tc.tile_pool(name="op", bufs=4) as op, \
             tc.tile_pool(name="p0", bufs=NKT) as p0pool, \
             tc.tile_pool(name="p1a", bufs=N1PH1) as p1apool:

            phase1_pools = [
                tc.tile_pool(name="const", bufs=1),
                tc.tile_pool(name="xtp", bufs=2),
                tc.tile_pool(name="vtmp", bufs=2),
            ]
            constp, xtp, vtmpp = [pl.__enter__() for pl in phase1_pools]
            # DMA completion is roughly fair-share over all in-flight
            # transfers (16 engines round-robin packets, ~0.37 GB/us
            # aggregate), so the start is fastest when as little as possible
            # is in flight: wk + quartered chunk 0 go first, everything else
            # after, and the xt pool is kept at bufs=2 so later chunk DMAs
            # are throttled by consumption instead of flooding the pool.
            wk_sb = constp.tile([P, E], f16, name="wk16")
            nc.sync.dma_start(out=wk_sb, in_=wk[:, :])
            x0 = xtp.tile([P, E * 4], f16, tag="xt", name="xt")
            for q4 in range(4):
                nc.sync.dma_start(out=x0[:, q4 * E:(q4 + 1) * E],
                                  in_=xt[0:P, q4 * E:(q4 + 1) * E])
            wv_sb = constp.tile([P, E], f16, name="wv16")
            nc.sync.dma_start(out=wv_sb, in_=wv[:, :])
            x1 = xtp.tile([P, E * 4], f16, tag="xt", name="xt")
            nc.sync.dma_start(out=x1[:, 0:E * 2], in_=xt[P:2 * P, 0:E * 2])
            wq_sb = constp.tile([P, E], f16, name="wq16")
            nc.sync.dma_start(out=wq_sb, in_=wq[:, :])
            nc.sync.dma_start(out=x1[:, E * 2:E * 4], in_=xt[P:2 * P, E * 2:E * 4])
            bias_sb = constp.tile([P, 3], f32, name="bias")
            nc.sync.dma_start(out=bias_sb, in_=bias[:, :])
            id16 = constp.tile([P, P], f16)
            nc.sync.dma_start(out=id16, in_=ident[:, :])
            xts = [x0, x1]
            bk_sb = bias_sb[:, 0:1]
            bv_sb = bias_sb[:, 1:2]
            bq_sb = bias_sb[:, 2:3]

            # persistent activations (all fp16)
            kT_sb = bigp.tile([P, S], f16)        # K^T  [d, s]
            qT_sb = bigp.tile([P, SQ], f16)       # Q^T  [d, q]
            v_all = bigp.tile([P, NKT, D + 1], f16)  # [k_local, kt, 128 V | ones]
            nc.vector.memset(v_all[:, :, D:D + 1], 1.0)

            def s_exp(sp_pool, p_pool, qb, kt, w=QBLK, qoff=0):
                sp = sp_pool.tile([P, w], f32, tag="sp", name="sp")
                for h in range(w // SC):
                    nc.tensor.matmul(sp[:, h * SC:(h + 1) * SC],
                                     kT_sb[:, kt * P:(kt + 1) * P],
                                     qT_sb[:, qb * QBLK + qoff + h * SC:
                                           qb * QBLK + qoff + (h + 1) * SC],
                                     start=True, stop=True)
                p_sb = p_pool.tile([P, w], f16, tag="p", name="p")
                nc.scalar.activation(p_sb, sp, AF.Exp, scale=SCALE)
                return p_sb

            # score tiles interleaved after each s-chunk's projections:
            # chunk sc makes kt 4sc..4sc+3 available; qb0 needs qT from chunks
            # 0-1 (ready sc>=1), qb1 needs chunks 2-3 (ready sc>=3).
            p1_kts = {1: [(0, k) for k in range(0, 4)],
                      2: [(0, k) for k in range(4, 10)],
                      3: [(0, k) for k in range(10, 16)],
                      4: [(0, k) for k in range(16, 20)] + [(1, k) for k in range(0, 2)],
                      5: [(0, k) for k in range(20, 24)] + [(1, k) for k in range(2, 5)],
                      6: [(0, k) for k in range(24, 28)] + [(1, k) for k in range(5, 8)],
                      7: [(0, k) for k in range(28, 32)] + [(1, k) for k in range(8, N1PH1)]}
            p_tiles = {}

            # ---------------- phase 1: QKV + all qb0 / first qb1 scores ----------------
            with tc.tile_pool(name="proj_ps", bufs=1, space="PSUM") as proj_ps, \
                 tc.tile_pool(name="vt_ps", bufs=1, space="PSUM") as vt_ps, \
                 tc.tile_pool(name="sp1_ps", bufs=2, space="PSUM") as sp1_ps:
                for sc in range(NSC):
                    if sc < 2:
                        x16 = xts[sc]
                    else:
                        x16 = xtp.tile([P, E * 4], f16, tag="xt", name="xt")
                        nc.sync.dma_start(out=x16, in_=xt[sc * P:(sc + 1) * P, :])
                    # sequential K, V, Q accumulation: kT's bias-add (and so
                    # the interleaved scores) fire 2x earlier than with
                    # per-ec interleaving of all three projections.
                    pk = proj_ps.tile([P, SC], f32, tag="pk", name="pk")
                    pv = proj_ps.tile([P, SC], f32, tag="pv", name="pv")
                    pq = proj_ps.tile([P, SC], f32, tag="pq", name="pq") if sc < NSC // 2 else None
                    tiles = list(p1_kts.get(sc, ()))
                    half = len(tiles) // 2
                    for dst, w_sb in ((pk, wk_sb), (pv, wv_sb), (pq, wq_sb)):
                        if dst is None:
                            continue
                        for ec in range(NEC):
                            nc.tensor.matmul(dst, w_sb[:, ec * P:(ec + 1) * P],
                                             x16[:, ec * SC:(ec + 1) * SC],
                                             start=(ec == 0), stop=(ec == NEC - 1))
                        if dst is pk:
                            # kT ready: start this chunk's first score tiles
                            # mid-chunk so ScalarE never runs dry
                            nc.vector.tensor_scalar_add(
                                kT_sb[:, sc * SC:(sc + 1) * SC], pk, bk_sb)
                            for qb, kt in tiles[:half]:
                                pool = p0pool if qb == 0 else p1apool
                                p_tiles[(qb, kt)] = s_exp(sp1_ps, pool, qb, kt)
                    if pq is not None:
                        nc.vector.tensor_scalar_add(qT_sb[:, sc * SC:(sc + 1) * SC], pq, bq_sb)
                    # V: bias add (f32 psum -> f16), PE transpose, pack into v_all
                    vtmp = vtmpp.tile([P, SC], f16, tag="vtmp", name="vtmp")
                    nc.vector.tensor_scalar_add(vtmp, pv, bv_sb)
                    vt = vt_ps.tile([P, SC], f16, tag="vt", name="vt")
                    for i in range(4):
                        nc.tensor.transpose(vt[:, i * P:(i + 1) * P],
                                            vtmp[:, i * P:(i + 1) * P],
                                            id16)
                    nc.vector.tensor_copy(
                        v_all[:, sc * 4:(sc + 1) * 4, 0:D],
                        vt[:, :].rearrange("p (b c) -> p b c", c=P))
                    for qb, kt in tiles[half:]:
                        pool = p0pool if qb == 0 else p1apool
                        p_tiles[(qb, kt)] = s_exp(sp1_ps, pool, qb, kt)

            # phase-1-only SBUF pools released: phase 2 needs the space for
            # the remaining qb=1 P tiles (full overlap of S/exp and P@V)
            for pl in reversed(phase1_pools):
                pl.__exit__(None, None, None)

            # ---------------- phase 2: remaining qb1 scores + both P@V sweeps ----------------
            with tc.tile_pool(name="pp", bufs=NKT - N1PH1) as pp, \
                 tc.tile_pool(name="sp_ps", bufs=2, space="PSUM") as sp_ps, \
                 tc.tile_pool(name="acc_ps", bufs=4, space="PSUM") as acc_ps:
                def sweep(qb, qs):
                    acc = acc_ps.tile([P, D + 1], f32, tag="acc", name="acc")
                    for kt in range(NKT):
                        nc.tensor.matmul(acc,
                                         p_tiles[(qb, kt)][:, qs * P:(qs + 1) * P],
                                         v_all[:, kt, :],
                                         start=(kt == 0), stop=(kt == NKT - 1))
                    rec = op.tile([P, 1], f32, tag="rec", name="rec")
                    nc.vector.reciprocal(rec, acc[:, D:D + 1])
                    o_sb = op.tile([P, D], f32, tag="o", name="o")
                    nc.vector.tensor_scalar_mul(o_sb, acc[:, 0:D], rec)
                    q0 = (qb * (QBLK // P) + qs) * P
                    nc.sync.dma_start(out=out[q0:q0 + P, :], in_=o_sb)

                # interleave the remaining qb1 score/exp tiles with qb0's
                # sweeps at a 2:1 ratio so the in-order PE stream never stalls
                # on the sp-pool rotation (exp 1147ns vs 2x426ns of scores).
                qs0 = 0
                for kt in range(N1PH1, NKT):
                    p_tiles[(1, kt)] = s_exp(sp_ps, pp, 1, kt)
                    if kt % 2 == 1 and qs0 < QBLK // P:
                        sweep(0, qs0)
                        qs0 += 1
                while qs0 < QBLK // P:
                    sweep(0, qs0)
                    qs0 += 1
                for qs in range(QBLK // P):
                    sweep(1, qs)
    nc.finalize()
    return nc


def _get_nc():
    if "nc" not in _CACHE:
        _CACHE["nc"] = _build_nc()
    return _CACHE["nc"]


def _prepack_w(w):
    # [E, D] f32 -> [p, ec*128+c] = W[ec*128+p, c], f16
    w = np.asarray(w, np.float32).astype(np.float16)
    return np.ascontiguousarray(
        w.reshape(NEC, P, D).transpose(1, 0, 2).reshape(P, E))


def _in_maps(x, Wq, bq, Wk, bk, Wv, bv):
    x = np.asarray(x, dtype=np.float32).astype(np.float16)
    bias = np.stack([np.asarray(b, np.float32).reshape(D) for b in (bk, bv, bq)],
                    axis=1)
    shared = {
        "wqp": _prepack_w(Wq),
        "wkp": _prepack_w(Wk),
        "wvp": _prepack_w(Wv),
        "bias": np.ascontiguousarray(bias),
        "ident": np.eye(P, dtype=np.float16),
    }
    maps = []
    for core in range(8):
        b, h = core // 2, core % 2
        xb = x[b] if h == 0 else np.concatenate([x[b, SQ:], x[b, :SQ]], axis=0)
        # [s, e] -> x^T [e, s] -> chunk layout [sc*128+p, ec*512+c]
        xT = xb.T  # [E, S]
        y = xT.reshape(NEC, P, NSC, SC).transpose(2, 1, 0, 3).reshape(NSC * P, NEC * SC)
        maps.append({"xt": np.ascontiguousarray(y), **shared})
    return maps


def _assemble(results):
    out = np.empty((4, S, D), dtype=np.float32)
    for core in range(8):
        b, h = core // 2, core % 2
        out[b, h * SQ:(h + 1) * SQ] = results[core]["out"]
    return out


def kernel(x, Wq, bq, Wk, bk, Wv, bv):
    from concourse.bass_utils import run_bass_kernel_spmd

    nc = _get_nc()
    res = run_bass_kernel_spmd(nc, _in_maps(x, Wq, bq, Wk, bk, Wv, bv),
                               core_ids=list(range(8)))
    return _assemble(res.results)


# revision 27
# speedup vs baseline: 1.0002x; 1.0002x over previous
"""Single-head attention kernel for Trainium2, SPMD over 8 NeuronCores.

Problem: x [4,4096,1024] f32 -> q/k/v = x@W+b (head 128) -> softmax(q k^T/sqrt(128)) @ v.
Sharding: core i handles batch i//2, query half i%2. Each core receives its
batch's x pre-transposed on the host to x^T [e, s] layout (and rotated so its
2048 queries are cols 0:2048; key order is irrelevant to softmax sums), so all
cores run one identical program and no on-chip transposes of x are needed.

Perf notes (from NTFF traces on this hardware):
- fp16 matmul is 1 cyc/row (fp32 is 4); everything is O(10) so the compute
  path runs fp16 with fp32 PSUM accumulation (measured ~5e-4 end-to-end).
- Host supplies x^T chunked as [sc*128+p, ec*512+c] and weights prepacked to
  [p, ec*128+c] f16 so every input is a single wide-row DMA.
- The kernel is PE-saturated end-to-end (~100us busy of ~118us): 90.7us of
  fp16 matmul columns + p-state ramp (~3us: 1.2GHz until 3us of GAP-FREE
  execution; any stall resets the clock) + power throttling (6-10us,
  run-to-run noise). Warmup matmuls don't help: the early DMA-paced stalls
  reset the ramp anyway.
- DMA completion is fair-share over in-flight transfers (16 engines
  round-robin packets, ~0.37 GB/us aggregate), so the critical first loads
  (wk, chunk 0 in quarters) are issued with nothing else in flight, and
  later chunk DMAs are throttled via the bufs=2 xt pool.
- exp on ScalarE costs ~(N+352)/1.2ns per instruction (~78us/core total):
  all 32 qb0 tiles plus the first 10 qb1 tiles are computed+exp'd inside
  phase 1 (starting mid-chunk, right after each chunk's K bias-add) so
  ScalarE saturates early; remaining qb1 score/exp tiles are interleaved
  2:1 with qb0's P@V sweeps in issue order.
- A pair-wise HBM AllReduce to split the duplicated K/V projection work was
  measured at ~45-50us for 1MB (plus ~7us extra teardown) and abandoned.
- P@V appends a ones-column to V so the softmax denominator lands in PSUM
  column 128 of each accumulator for free.
- Tile dependency hazard: a tile-slice READ issued before the program-order
  WRITE of that slice gets no dependency edge (reads stale data silently).
"""

import sys

if "/opt/trn_rl_repo" not in sys.path:
    sys.path.insert(0, "/opt/trn_rl_repo")

import numpy as np

P = 128          # partitions
S = 4096         # sequence length
E = 1024         # n_embd
D = 128          # head size
SQ = 2048        # queries per core
SC = 512         # s-processing chunk (phase 1)
NSC = S // SC    # 8
NEC = E // P     # 8
NKT = S // P     # 32 key tiles
QBLK = 1024      # query block (ACT instruction width)
NQB = SQ // QBLK # 2
N1PH1 = 10       # qb1 score tiles computed in phase 1

SCALE = 1.0 / float(np.sqrt(D))

_CACHE = {}


def _build_nc():
    import concourse.mybir as mybir
    import concourse.tile as tile
    from concourse import bacc

    f32 = mybir.dt.float32
    f16 = mybir.dt.float16
    AF = mybir.ActivationFunctionType

    nc = bacc.Bacc(None, target_bir_lowering=False)
    # xt[sc*128+p, ec*512+c] = x^T[ec*128+p, sc*512+c] (host pre-chunked)
    xt = nc.dram_tensor("xt", [S // SC * P, NEC * SC], f16, kind="ExternalInput")
    # weights host-prepacked to [p, ec*128+c] = W[ec*128+p, c], f16
    wq = nc.dram_tensor("wqp", [P, E], f16, kind="ExternalInput")
    wk = nc.dram_tensor("wkp", [P, E], f16, kind="ExternalInput")
    wv = nc.dram_tensor("wvp", [P, E], f16, kind="ExternalInput")
    bias = nc.dram_tensor("bias", [P, 3], f32, kind="ExternalInput")  # [bk|bv|bq]
    ident = nc.dram_tensor("ident", [P, P], f16, kind="ExternalInput")
    out = nc.dram_tensor("out", [SQ, D], f32, kind="ExternalOutput")

    with tile.TileContext(nc) as tc:
        with tc.tile_pool(name="big", bufs=1) as bigp, \
             tc.tile_pool(name="op", bufs=4) as op, \
             tc.tile_pool(name="p0", bufs=NKT) as p0pool, \
             tc.tile_pool(name="p1a", bufs=N1PH1) as p1apool:

            phase1_pools = [
                tc.tile_pool(name="const", bufs=1),
                tc.tile_pool(name="xtp", bufs=2),
                tc.tile_pool(name="vtmp", bufs=2),
            ]
            constp, xtp, vtmpp = [pl.__enter__() for pl in phase1_pools]
            # DMA completion is roughly fair-share over all in-flight
            # transfers (16 engines round-robin packets, ~0.37 GB/us
            # aggregate), so the start is fastest when as little as possible
            # is in flight: wk + quartered chunk 0 go first, everything else
            # after, and the xt pool is kept at bufs=2 so later chunk DMAs
            # are throttled by consumption instead of flooding the pool.
            wk_sb = constp.tile([P, E], f16, name="wk16")
            nc.sync.dma_start(out=wk_sb, in_=wk[:, :])
            x0 = xtp.tile([P, E * 4], f16, tag="xt", name="xt")
            for q4 in range(4):
                nc.sync.dma_start(out=x0[:, q4 * E:(q4 + 1) * E],
                                  in_=xt[0:P, q4 * E:(q4 + 1) * E])
            wv_sb = constp.tile([P, E], f16, name="wv16")
            nc.sync.dma_start(out=wv_sb, in_=wv[:, :])
            wq_sb = constp.tile([P, E], f16, name="wq16")
            nc.sync.dma_start(out=wq_sb, in_=wq[:, :])
            x1 = xtp.tile([P, E * 4], f16, tag="xt", name="xt")
            nc.sync.dma_start(out=x1, in_=xt[P:2 * P, :])
            bias_sb = constp.tile([P, 3], f32, name="bias")
            nc.sync.dma_start(out=bias_sb, in_=bias[:, :])
            id16 = constp.tile([P, P], f16)
            nc.sync.dma_start(out=id16, in_=ident[:, :])
            xts = [x0, x1]
            bk_sb = bias_sb[:, 0:1]
            bv_sb = bias_sb[:, 1:2]
            bq_sb = bias_sb[:, 2:3]

            # persistent activations (all fp16)
            kT_sb = bigp.tile([P, S], f16)        # K^T  [d, s]
            qT_sb = bigp.tile([P, SQ], f16)       # Q^T  [d, q]
            v_all = bigp.tile([P, NKT, D + 1], f16)  # [k_local, kt, 128 V | ones]
            nc.vector.memset(v_all[:, :, D:D + 1], 1.0)

            def s_exp(sp_pool, p_pool, qb, kt, w=QBLK, qoff=0):
                sp = sp_pool.tile([P, w], f32, tag="sp", name="sp")
                for h in range(w // SC):
                    nc.tensor.matmul(sp[:, h * SC:(h + 1) * SC],
                                     kT_sb[:, kt * P:(kt + 1) * P],
                                     qT_sb[:, qb * QBLK + qoff + h * SC:
                                           qb * QBLK + qoff + (h + 1) * SC],
                                     start=True, stop=True)
                p_sb = p_pool.tile([P, w], f16, tag="p", name="p")
                nc.scalar.activation(p_sb, sp, AF.Exp, scale=SCALE)
                return p_sb

            # score tiles interleaved after each s-chunk's projections:
            # chunk sc makes kt 4sc..4sc+3 available; qb0 needs qT from chunks
            # 0-1 (ready sc>=1), qb1 needs chunks 2-3 (ready sc>=3).
            p1_kts = {1: [(0, k) for k in range(0, 4)],
                      2: [(0, k) for k in range(4, 10)],
                      3: [(0, k) for k in range(10, 16)],
                      4: [(0, k) for k in range(16, 20)] + [(1, k) for k in range(0, 2)],
                      5: [(0, k) for k in range(20, 24)] + [(1, k) for k in range(2, 5)],
                      6: [(0, k) for k in range(24, 28)] + [(1, k) for k in range(5, 8)],
                      7: [(0, k) for k in range(28, 32)] + [(1, k) for k in range(8, N1PH1)]}
            p_tiles = {}

            # ---------------- phase 1: QKV + all qb0 / first qb1 scores ----------------
            with tc.tile_pool(name="proj_ps", bufs=1, space="PSUM") as proj_ps, \
                 tc.tile_pool(name="vt_ps", bufs=1, space="PSUM") as vt_ps, \
                 tc.tile_pool(name="sp1_ps", bufs=2, space="PSUM") as sp1_ps:
                for sc in range(NSC):
                    if sc < 2:
                        x16 = xts[sc]
                    else:
                        x16 = xtp.tile([P, E * 4], f16, tag="xt", name="xt")
                        nc.sync.dma_start(out=x16, in_=xt[sc * P:(sc + 1) * P, :])
                    # sequential K, V, Q accumulation: kT's bias-add (and so
                    # the interleaved scores) fire 2x earlier than with
                    # per-ec interleaving of all three projections.
                    pk = proj_ps.tile([P, SC], f32, tag="pk", name="pk")
                    pv = proj_ps.tile([P, SC], f32, tag="pv", name="pv")
                    pq = proj_ps.tile([P, SC], f32, tag="pq", name="pq") if sc < NSC // 2 else None
                    tiles = list(p1_kts.get(sc, ()))
                    # sc=1's tiles read qT cols written by THIS chunk's Q
                    # projection, so they must go after it (end of chunk)
                    half = len(tiles) // 2 if sc >= 2 else 0
                    for dst, w_sb in ((pk, wk_sb), (pv, wv_sb), (pq, wq_sb)):
                        if dst is None:
                            continue
                        for ec in range(NEC):
                            nc.tensor.matmul(dst, w_sb[:, ec * P:(ec + 1) * P],
                                             x16[:, ec * SC:(ec + 1) * SC],
                                             start=(ec == 0), stop=(ec == NEC - 1))
                        if dst is pk:
                            # kT ready: start this chunk's first score tiles
                            # mid-chunk so ScalarE never runs dry
                            nc.vector.tensor_scalar_add(
                                kT_sb[:, sc * SC:(sc + 1) * SC], pk, bk_sb)
                            for qb, kt in tiles[:half]:
                                pool = p0pool if qb == 0 else p1apool
                                p_tiles[(qb, kt)] = s_exp(sp1_ps, pool, qb, kt)
                    if pq is not None:
                        nc.vector.tensor_scalar_add(qT_sb[:, sc * SC:(sc + 1) * SC], pq, bq_sb)
                    # V: bias add (f32 psum -> f16), PE transpose, pack into v_all
                    vtmp = vtmpp.tile([P, SC], f16, tag="vtmp", name="vtmp")
                    nc.vector.tensor_scalar_add(vtmp, pv, bv_sb)
                    vt = vt_ps.tile([P, SC], f16, tag="vt", name="vt")
                    for i in range(4):
                        nc.tensor.transpose(vt[:, i * P:(i + 1) * P],
                                            vtmp[:, i * P:(i + 1) * P],
                                            id16)
                    nc.vector.tensor_copy(
                        v_all[:, sc * 4:(sc + 1) * 4, 0:D],
                        vt[:, :].rearrange("p (b c) -> p b c", c=P))
                    for qb, kt in tiles[half:]:
                        pool = p0pool if qb == 0 else p1apool
                        p_tiles[(qb, kt)] = s_exp(sp1_ps, pool, qb, kt)

            # phase-1-only SBUF pools released: phase 2 needs the space for
            # the remaining qb=1 P tiles (full overlap of S/exp and P@V)
            for pl in reversed(phase1_pools):
                pl.__exit__(None, None, None)

            # ---------------- phase 2: remaining qb1 scores + both P@V sweeps ----------------
            with tc.tile_pool(name="pp", bufs=NKT - N1PH1) as pp, \
                 tc.tile_pool(name="sp_ps", bufs=2, space="PSUM") as sp_ps, \
                 tc.tile_pool(name="acc_ps", bufs=4, space="PSUM") as acc_ps:
                def sweep(qb, qs):
                    acc = acc_ps.tile([P, D + 1], f32, tag="acc", name="acc")
                    for kt in range(NKT):
                        nc.tensor.matmul(acc,
                                         p_tiles[(qb, kt)][:, qs * P:(qs + 1) * P],
                                         v_all[:, kt, :],
                                         start=(kt == 0), stop=(kt == NKT - 1))
                    rec = op.tile([P, 1], f32, tag="rec", name="rec")
                    nc.vector.reciprocal(rec, acc[:, D:D + 1])
                    o_sb = op.tile([P, D], f32, tag="o", name="o")
                    nc.vector.tensor_scalar_mul(o_sb, acc[:, 0:D], rec)
                    q0 = (qb * (QBLK // P) + qs) * P
                    nc.sync.dma_start(out=out[q0:q0 + P, :], in_=o_sb)

                # interleave the remaining qb1 score/exp tiles with qb0's
                # sweeps at a 2:1 ratio so the in-order PE stream never stalls
                # on the sp-pool rotation (exp 1147ns vs 2x426ns of scores).
                qs0 = 0
                for kt in range(N1PH1, NKT):
                    p_tiles[(1, kt)] = s_exp(sp_ps, pp, 1, kt)
                    if kt % 2 == 1 and qs0 < QBLK // P:
                        sweep(0, qs0)
                        qs0 += 1
                while qs0 < QBLK // P:
                    sweep(0, qs0)
                    qs0 += 1
                for qs in range(QBLK // P):
                    sweep(1, qs)
    nc.finalize()
    return nc


def _get_nc():
    if "nc" not in _CACHE:
        _CACHE["nc"] = _build_nc()
    return _CACHE["nc"]


def _prepack_w(w):
    # [E, D] f32 -> [p, ec*128+c] = W[ec*128+p, c], f16
    w = np.asarray(w, np.float32).astype(np.float16)
    return np.ascontiguousarray(
        w.reshape(NEC, P, D).transpose(1, 0, 2).reshape(P, E))


def _in_maps(x, Wq, bq, Wk, bk, Wv, bv):
    x = np.asarray(x, dtype=np.float32).astype(np.float16)
    bias = np.stack([np.asarray(b, np.float32).reshape(D) for b in (bk, bv, bq)],
                    axis=1)
    shared = {
        "wqp": _prepack_w(Wq),
        "wkp": _prepack_w(Wk),
        "wvp": _prepack_w(Wv),
        "bias": np.ascontiguousarray(bias),
        "ident": np.eye(P, dtype=np.float16),
    }
    maps = []
    for core in range(8):
        b, h = core // 2, core % 2
        xb = x[b] if h == 0 else np.concatenate([x[b, SQ:], x[b, :SQ]], axis=0)
        # [s, e] -> x^T [e, s] -> chunk layout [sc*128+p, ec*512+c]
        xT = xb.T  # [E, S]
        y = xT.reshape(NEC, P, NSC, SC).transpose(2, 1, 0, 3).reshape(NSC * P, NEC * SC)
        maps.append({"xt": np.ascontiguousarray(y), **shared})
    return maps


def _assemble(results):
    out = np.empty((4, S, D), dtype=np.float32)
    for core in range(8):
        b, h = core // 2, core % 2
        out[b, h * SQ:(h + 1) * SQ] = results[core]["out"]
    return out


def kernel(x, Wq, bq, Wk, bk, Wv, bv):
    from concourse.bass_utils import run_bass_kernel_spmd

    nc = _get_nc()
    res = run_bass_kernel_spmd(nc, _in_maps(x, Wq, bq, Wk, bk, Wv, bv),
                               core_ids=list(range(8)))
    return _assemble(res.results)
